# revision 26
# baseline (speedup 1.0000x reference)
"""ECPGLinear (ternary-quantized linear) Bass kernel for 8 TRN2 NeuronCores.

Computes out = x @ W.T where W = dequant(ternary, per-group scales),
group_size=128 along in_features.

Sharding: data-parallel over the 8192 (batch*seq) tokens — each core takes
1024 rows of x and the full weight matrix; no collectives, the host
concatenates the 8 output shards.

Per-core schedule (dequant + matmul on-device, fp16 compute):
  - X^T shard resident in SBUF (cast f32 -> fp16 during the load DMA).
  - Per (n-chunk, k-tile): DMA a [128 x 512] ternary^T tile (fp16 ±1/0)
    and the matching replicated-scale tile, DVE-multiply -> dequantized
    W^T tile, then 8 matmuls (one per m-tile) accumulate into 8 PSUM
    banks over the 32 k-tiles.
  - ACT evicts PSUM to SBUF and its HWDGE queue stores to DRAM.

Host prep is layout-only: transpose/shard/dtype-cast and replication of
the per-group scales across the 128 partitions. Since ternary is in
{-1,0,1}, rounding scales to fp16 on the host is bit-identical to
dequantizing in fp32 on-device and rounding: fp16(t*s) == t*fp16(s).
"""
import functools
import numpy as np

OUT_F = 4096
IN_F = 4096
B, S = 4, 2048
M_TOT = B * S             # 8192 tokens
NCORES = 8
M_CORE = M_TOT // NCORES  # 1024 tokens per core
KT = IN_F // 128          # 32 contraction tiles
NCH = OUT_F // 512        # 8 output chunks of 512
MT = M_CORE // 128        # 8 m-tiles per core


@functools.lru_cache(maxsize=1)
def _build():
    from concourse import bacc
    import concourse.mybir as mybir
    import concourse.tile as tile

    f32 = mybir.dt.float32
    f16 = mybir.dt.float16

    nc = bacc.Bacc("TRN2", target_bir_lowering=False, debug=False,
                   num_devices=NCORES)
    xt = nc.dram_tensor("xt", [IN_F, M_CORE], f16, kind="ExternalInput")
    tt = nc.dram_tensor("tt", [IN_F, OUT_F], mybir.dt.int8, kind="ExternalInput")
    # scales pre-replicated across partitions: [KT, 128, OUT_F]
    sc = nc.dram_tensor("sc", [KT, 128, OUT_F], f16, kind="ExternalInput")

    out = nc.dram_tensor("out", [M_CORE, OUT_F], f32, kind="ExternalOutput")

    with tile.TileContext(nc) as tc:
        with (
            tc.tile_pool(name="xres", bufs=1) as xres_pool,
            tc.tile_pool(name="scb", bufs=8) as scb_pool,
            tc.tile_pool(name="tern", bufs=8) as tern_pool,
            tc.tile_pool(name="wd", bufs=8) as wd_pool,
            tc.tile_pool(name="ost", bufs=12) as ost_pool,
            tc.tile_pool(name="psum", bufs=8, space="PSUM") as psum_pool,
        ):
            # Resident X^T: [128 part, KT, M_CORE]; tile kt is
            # loaded inside the n=0 loop right before its first use.
            xres = xres_pool.tile([128, KT, M_CORE], f16)

            # PE warmup: keep the HAM busy while X^T/first W tiles load.
            warm_l = scb_pool.tile([128, 128], f16, name="warm_l",
                                   tag="warm")
            warm_r = tern_pool.tile([128, 512], f16, name="warm_r",
                                    tag="warm_r")
            nc.vector.memset(warm_l[:], 0.0)
            nc.vector.memset(warm_r[:], 0.0)
            warm_ps = psum_pool.tile([128, 512], f32, name="warm_ps",
                                     tag="ps")
            for _ in range(10):
                nc.tensor.matmul(warm_ps[:], warm_l[:], warm_r[:],
                                 start=True, stop=True)

            for n in range(NCH):
                o0 = n * 512
                psums = [psum_pool.tile([128, 512], f32, name=f"ps{n}_{m}",
                                        tag="ps")
                         for m in range(MT)]
                for kt in range(KT):
                    if n == 0:
                        nc.sync.dma_start(xres[:, kt, :],
                                          xt[kt * 128:(kt + 1) * 128, :])
                    scb = scb_pool.tile([128, 512], f16,
                                        name=f"scb{n}_{kt}", tag="scb")
                    nc.sync.dma_start(scb[:], sc[kt, :, o0:o0 + 512])
                    tern = tern_pool.tile([128, 512], mybir.dt.int8,
                                          name=f"tern{n}_{kt}", tag="tern")
                    nc.sync.dma_start(
                        tern[:], tt[kt * 128:(kt + 1) * 128, o0:o0 + 512])
                    wd = wd_pool.tile([128, 512], f16,
                                      name=f"wd{n}_{kt}", tag="wd")
                    nc.vector.tensor_mul(wd[:], tern[:], scb[:])
                    for m in range(MT):
                        nc.tensor.matmul(
                            psums[m][:],
                            xres[:, kt, m * 128:(m + 1) * 128],
                            wd[:],
                            start=(kt == 0),
                            stop=(kt == KT - 1),
                        )
                last = n == NCH - 1
                for m in range(MT):
                    ost = ost_pool.tile([128, 512], f32,
                                        name=f"ost{n}_{m}", tag="ost")
                    # DVE PSUM-read copies are ~2x faster than ACT and
                    # DVE has slack; route the bank that gates the next
                    # chunk's first matmul (m=0) there, and split the whole
                    # final chunk across both engines to shorten the tail.
                    if m == 0 or (last and m % 2 == 0):
                        nc.vector.tensor_copy(ost[:], psums[m][:])
                    else:
                        nc.scalar.copy(ost[:], psums[m][:])
                    dma = nc.sync if last else nc.scalar
                    dma.dma_start(
                        out[m * 128:(m + 1) * 128, o0:o0 + 512], ost[:])

    nc.compile()
    return nc


def kernel(x: np.ndarray, ternary: np.ndarray, scales: np.ndarray,
           _trace: bool = False):
    from concourse.bass_utils import run_bass_kernel_spmd

    nc = _build()

    x = np.asarray(x)
    ternary = np.asarray(ternary)
    scales = np.asarray(scales)

    xf = x.reshape(M_TOT, IN_F)
    ttm = np.ascontiguousarray(ternary.T.astype(np.int8))
    # scales as [KT, OUT_F] (sc[kt, o] = scales[o*KT + kt]), replicated
    # across the 128 partitions: [KT, 128, OUT_F]
    scm = np.ascontiguousarray(scales.reshape(OUT_F, KT).T.astype(np.float16))
    scr = np.ascontiguousarray(
        np.broadcast_to(scm[:, None, :], (KT, 128, OUT_F)))

    in_maps = []
    for c in range(NCORES):
        xc = np.ascontiguousarray(
            xf[c * M_CORE:(c + 1) * M_CORE, :].T.astype(np.float16))
        in_maps.append({"xt": xc, "tt": ttm, "sc": scr})

    res = run_bass_kernel_spmd(nc, in_maps, list(range(NCORES)),
                               trace=_trace)
    outs = [res.results[c]["out"] for c in range(NCORES)]
    full = np.concatenate(outs, axis=0).reshape(B, S, OUT_F)
    if _trace:
        kernel.last_results = res
    return full


kernel.last_results = None


# revision 27
# speedup vs baseline: 1.0410x; 1.0410x over previous
"""ECPGLinear (ternary-quantized linear) Bass kernel for 8 TRN2 NeuronCores.

Computes out = x @ W.T where W = dequant(ternary, per-group scales),
group_size=128 along in_features.

Sharding: data-parallel over the 8192 (batch*seq) tokens — each core takes
1024 rows of x and the full weight matrix; no collectives, the host
concatenates the 8 output shards.

Per-core schedule (dequant + matmul on-device, fp16 compute):
  - X^T shard resident in SBUF (cast f32 -> fp16 during the load DMA).
  - Per (n-chunk, k-tile): DMA a [128 x 512] ternary^T tile (fp16 ±1/0)
    and the matching replicated-scale tile, DVE-multiply -> dequantized
    W^T tile, then 8 matmuls (one per m-tile) accumulate into 8 PSUM
    banks over the 32 k-tiles.
  - ACT evicts PSUM to SBUF and its HWDGE queue stores to DRAM.

Host prep is layout-only: transpose/shard/dtype-cast and replication of
the per-group scales across the 128 partitions. Since ternary is in
{-1,0,1}, rounding scales to fp16 on the host is bit-identical to
dequantizing in fp32 on-device and rounding: fp16(t*s) == t*fp16(s).
"""
import functools
import numpy as np

OUT_F = 4096
IN_F = 4096
B, S = 4, 2048
M_TOT = B * S             # 8192 tokens
NCORES = 8
M_CORE = M_TOT // NCORES  # 1024 tokens per core
KT = IN_F // 128          # 32 contraction tiles
NCH = OUT_F // 512        # 8 output chunks of 512
MT = M_CORE // 128        # 8 m-tiles per core


@functools.lru_cache(maxsize=1)
def _build():
    from concourse import bacc
    import concourse.mybir as mybir
    import concourse.tile as tile

    f32 = mybir.dt.float32
    f16 = mybir.dt.float16

    nc = bacc.Bacc("TRN2", target_bir_lowering=False, debug=False,
                   num_devices=NCORES)
    xt = nc.dram_tensor("xt", [IN_F, M_CORE], f16, kind="ExternalInput")
    tt = nc.dram_tensor("tt", [IN_F, OUT_F], mybir.dt.int8, kind="ExternalInput")
    # scales pre-replicated across partitions: [KT, 128, OUT_F]
    sc = nc.dram_tensor("sc", [KT, 128, OUT_F], f16, kind="ExternalInput")

    out = nc.dram_tensor("out", [M_CORE, OUT_F], f32, kind="ExternalOutput")

    with tile.TileContext(nc) as tc:
        with (
            tc.tile_pool(name="xres", bufs=1) as xres_pool,
            tc.tile_pool(name="scb", bufs=8) as scb_pool,
            tc.tile_pool(name="tern", bufs=8) as tern_pool,
            tc.tile_pool(name="wd", bufs=8) as wd_pool,
            tc.tile_pool(name="ost", bufs=12) as ost_pool,
            tc.tile_pool(name="psum", bufs=8, space="PSUM") as psum_pool,
        ):
            # Resident X^T: [128 part, KT, M_CORE]; tile kt is
            # loaded inside the n=0 loop right before its first use.
            xres = xres_pool.tile([128, KT, M_CORE], f16)

            # PE warmup: keep the HAM busy while X^T/first W tiles load.
            warm_l = scb_pool.tile([128, 128], f16, name="warm_l",
                                   tag="warm")
            warm_r = tern_pool.tile([128, 512], f16, name="warm_r",
                                    tag="warm_r")
            nc.vector.memset(warm_l[:], 0.0)
            nc.vector.memset(warm_r[:], 0.0)
            warm_ps = psum_pool.tile([128, 512], f32, name="warm_ps",
                                     tag="ps")
            for _ in range(10):
                nc.tensor.matmul(warm_ps[:], warm_l[:], warm_r[:],
                                 start=True, stop=True)

            for n in range(NCH):
                o0 = n * 512
                psums = [psum_pool.tile([128, 512], f32, name=f"ps{n}_{m}",
                                        tag="ps")
                         for m in range(MT)]
                for kt in range(KT):
                    if n == 0:
                        nc.sync.dma_start(xres[:, kt, :],
                                          xt[kt * 128:(kt + 1) * 128, :])
                    scb = scb_pool.tile([128, 512], f16,
                                        name=f"scb{n}_{kt}", tag="scb")
                    nc.sync.dma_start(scb[:], sc[kt, :, o0:o0 + 512])
                    tern = tern_pool.tile([128, 512], mybir.dt.int8,
                                          name=f"tern{n}_{kt}", tag="tern")
                    nc.gpsimd.dma_start(
                        tern[:], tt[kt * 128:(kt + 1) * 128, o0:o0 + 512])
                    wd = wd_pool.tile([128, 512], f16,
                                      name=f"wd{n}_{kt}", tag="wd")
                    nc.vector.tensor_mul(wd[:], tern[:], scb[:])
                    for m in range(MT):
                        nc.tensor.matmul(
                            psums[m][:],
                            xres[:, kt, m * 128:(m + 1) * 128],
                            wd[:],
                            start=(kt == 0),
                            stop=(kt == KT - 1),
                        )
                last = n == NCH - 1
                for m in range(MT):
                    ost = ost_pool.tile([128, 512], f32,
                                        name=f"ost{n}_{m}", tag="ost")
                    # Final chunk: DVE is idle (no more dequant) and its
                    # PSUM-read copies are ~2x faster than ACT; split the
                    # copy/store across engines to shorten the tail chain.
                    if last and m % 2 == 0:
                        nc.vector.tensor_copy(ost[:], psums[m][:])
                    else:
                        nc.scalar.copy(ost[:], psums[m][:])
                    dma = nc.sync if last else nc.scalar
                    dma.dma_start(
                        out[m * 128:(m + 1) * 128, o0:o0 + 512], ost[:])

    nc.compile()
    return nc


def kernel(x: np.ndarray, ternary: np.ndarray, scales: np.ndarray,
           _trace: bool = False):
    from concourse.bass_utils import run_bass_kernel_spmd

    nc = _build()

    x = np.asarray(x)
    ternary = np.asarray(ternary)
    scales = np.asarray(scales)

    xf = x.reshape(M_TOT, IN_F)
    ttm = np.ascontiguousarray(ternary.T.astype(np.int8))
    # scales as [KT, OUT_F] (sc[kt, o] = scales[o*KT + kt]), replicated
    # across the 128 partitions: [KT, 128, OUT_F]
    scm = np.ascontiguousarray(scales.reshape(OUT_F, KT).T.astype(np.float16))
    scr = np.ascontiguousarray(
        np.broadcast_to(scm[:, None, :], (KT, 128, OUT_F)))

    in_maps = []
    for c in range(NCORES):
        xc = np.ascontiguousarray(
            xf[c * M_CORE:(c + 1) * M_CORE, :].T.astype(np.float16))
        in_maps.append({"xt": xc, "tt": ttm, "sc": scr})

    res = run_bass_kernel_spmd(nc, in_maps, list(range(NCORES)),
                               trace=_trace)
    outs = [res.results[c]["out"] for c in range(NCORES)]
    full = np.concatenate(outs, axis=0).reshape(B, S, OUT_F)
    if _trace:
        kernel.last_results = res
    return full


kernel.last_results = None
